# revision 52
# baseline (speedup 1.0000x reference)
"""GCN inference kernel (y = D^-1/2 A D^-1/2 (x @ W.T)) on 8 Trainium2 NeuronCores.

Strategy (full inputs in, full output out; sharded internally):
  - Destination nodes are sharded across the 8 cores (12500 dsts each);
    edges are owned by the core that owns their dst, so the segment-sum is
    core-local (per the sharding hint).
  - Phase A (per core, replicated): compute the scaled projection table
    h~[n] = dinv[n] * (x[n] @ W.T) with PE matmuls and write it to an HBM
    table laid out in gather-order.
  - Phase B (per core): SWDGE dma_gather streams h~[src] rows (256B each)
    for the core's dst-sorted edge list into SBUF; a one-hot selection
    matrix B (built on DVE from dst-local ids vs an iota row) turns the
    segment-sum into PE matmuls accumulated in PSUM per 128-dst tile;
    a final per-dst dinv scale lands y.
  - All data-dependent structure (edge sort, padding, gather indices,
    one-hot ids, uniform per-core slice schedule) is prepared host-side in
    numpy; the device program is identical on all 8 cores (SPMD), only the
    per-core input arrays differ.
"""

import math
from dataclasses import dataclass, field

import numpy as np

import concourse.bacc as bacc
import concourse.bass as bass
import concourse.mybir as mybir
import concourse.tile as tile
from concourse import library_config
from concourse.bass_utils import run_bass_kernel_spmd

P = 128  # SBUF partitions
FIN = 128
FOUT = 64


@dataclass
class Prm:
    N: int = 100000  # nodes
    C: int = 8  # cores
    WG: int = 512  # nodes per phase-A write group
    BKCAP: int = 25600  # table rows per gather bucket (multiple of WG, <= 32767)
    SWD: int = 512  # dst nodes per superwindow (TPSW * P)
    S_CAP: int = 24  # max slices per dma_gather call (pipelining granularity)
    J: int = field(init=False)
    NS: int = field(init=False)  # dst shard size per core
    N2: int = field(init=False)  # padded node count (multiple of WG)
    NG: int = field(init=False)  # phase-A write groups
    NGpc: int = field(init=False)  # phase-A write groups per core
    NBK: int = field(init=False)  # gather buckets
    BR: int = field(init=False)  # rows per bucket incl. leading zero row
    TBLR: int = field(init=False)  # total table rows
    TPSW: int = field(init=False)  # dst tiles per superwindow
    NSW: int = field(init=False)  # superwindows per core

    def __post_init__(self):
        assert self.WG % P == 0 and self.BKCAP % self.WG == 0
        assert self.BKCAP <= 32767
        assert self.SWD % P == 0
        assert self.N % self.C == 0
        self.J = self.WG // P
        self.NS = self.N // self.C
        # N2: multiple of C*WG so phase A shards evenly across cores
        blk = self.C * self.WG
        self.N2 = ((self.N + blk - 1) // blk) * blk
        self.NG = self.N2 // self.WG
        self.NGpc = self.NG // self.C  # phase-A write groups per core
        self.NBK = (self.N2 + self.BKCAP - 1) // self.BKCAP
        self.BR = self.BKCAP  # rows per bucket (no zero row needed)
        self.TBLR = self.N2
        self.TPSW = self.SWD // P
        self.NSW = (self.NS + self.SWD - 1) // self.SWD


def _rmap(prm, n):
    """node id -> table slot (before per-bucket zero-row shift)."""
    return prm.WG * (n // prm.WG) + prm.J * (n % P) + (n % prm.WG) // P


def _wrap_idx(vals16):
    """[K] int16 (K % 128 == 0) -> [128, K//16] wrapped+replicated layout."""
    k = vals16.shape[0]
    w16 = vals16.reshape(k // 16, 16).T  # [16, K/16]
    return np.tile(w16, (8, 1))  # [128, K/16]


@dataclass
class CallMeta:
    sw: int
    bk: int
    k: int  # call index within its (sw, bk) section
    S: int  # slices in this call (one dma_gather per call)
    icol: int  # column offset into gidx array (8 * slice offset)
    scol: int  # column offset into dstl array (slice offset)


def _schedule(prm, n_sl_u):
    """Uniform (core-independent) schedule from the padded slice counts.

    Each (sw, bk) section is chunked into gather calls of <= S_CAP slices.
    Matmuls are emitted t-major per sw (all of a dst-tile's slices
    consecutive across buckets) so PSUM accumulation groups never
    interleave.
    Returns (calls, mms_by_sw, icol, scol).
    mms_by_sw[sw] = list of (bk, s_in_section, t, start, stop); the
    consumer maps s_in_section -> (call k = s // S_CAP, local s % S_CAP).
    """
    calls = []
    mms_by_sw = []
    icol = 0
    scol = 0
    for sw in range(prm.NSW):
        for bk in range(prm.NBK):
            nsl = sum(int(n_sl_u[sw][bk][t]) for t in range(prm.TPSW))
            for k, a in enumerate(range(0, nsl, prm.S_CAP)):
                S = min(prm.S_CAP, nsl - a)
                calls.append(CallMeta(sw, bk, k, S, icol, scol))
                icol += 8 * S
                scol += S
        # bucket-major matmul order: PE starts as soon as bucket 0's gather
        # lands. Each dst-tile t accumulates into its own PSUM tensor, so
        # accumulation groups may stay open across buckets.
        mms = []
        seen = [0] * prm.TPSW
        tot = [
            sum(int(n_sl_u[sw][bk][t]) for bk in range(prm.NBK))
            for t in range(prm.TPSW)
        ]
        for bk in range(prm.NBK):
            s0 = 0
            for t in range(prm.TPSW):
                for _ in range(int(n_sl_u[sw][bk][t])):
                    mms.append(
                        (bk, s0, t, seen[t] == 0, seen[t] == tot[t] - 1)
                    )
                    seen[t] += 1
                    s0 += 1
        mms_by_sw.append(mms)
    return calls, mms_by_sw, icol, scol


def _host_prep(x, edge_index, W, prm):
    N, C, NS = prm.N, prm.C, prm.NS
    src = np.asarray(edge_index[0], dtype=np.int64).astype(np.int32)
    dst = np.asarray(edge_index[1], dtype=np.int64).astype(np.int32)
    x = np.asarray(x, dtype=np.float32)
    W = np.asarray(W, dtype=np.float32)

    deg = np.bincount(dst, minlength=N).astype(np.float64)
    dinv = np.where(deg > 0, 1.0 / np.sqrt(np.maximum(deg, 1.0)), 0.0).astype(
        np.float32
    )

    # gather-order node map
    r_of = _rmap(prm, np.arange(N, dtype=np.int64)).astype(np.int64)
    bk_of = (r_of // prm.BKCAP).astype(np.int32)
    rel_of = (r_of % prm.BKCAP).astype(np.int16)

    # per-edge attributes
    core_e = dst // NS
    edl = dst - core_e * NS
    sw_e = edl // prm.SWD
    t_e = (edl % prm.SWD) // P
    q_e = (edl % P).astype(np.float32)
    bk_e = bk_of[src]
    rel_e = rel_of[src]

    # per-core sorted cell structure
    ncell = prm.NSW * prm.NBK * prm.TPSW
    counts = np.zeros((C, ncell), dtype=np.int64)
    percore = []
    for c in range(C):
        m = core_e == c
        order = np.lexsort((edl[m], t_e[m], bk_e[m], sw_e[m]))
        cell = (sw_e[m] * prm.NBK + bk_e[m]) * prm.TPSW + t_e[m]
        counts[c] = np.bincount(cell, minlength=ncell)
        percore.append(
            {
                "rel": rel_e[m][order],
                "q": q_e[m][order],
                "cell": cell[order],
            }
        )

    # uniform slice counts, >= 1 for in-range (sw, t) on bucket 0
    n_sl_u = np.zeros((prm.NSW, prm.NBK, prm.TPSW), dtype=np.int64)
    cmax = counts.max(axis=0).reshape(prm.NSW, prm.NBK, prm.TPSW)
    n_sl_u[:] = (cmax + P - 1) // P
    for sw in range(prm.NSW):
        ntile = min(prm.TPSW, max(0, -(-(NS - sw * prm.SWD) // P)))
        for t in range(ntile):
            if n_sl_u[sw, :, t].sum() == 0:
                n_sl_u[sw, 0, t] = 1

    calls, mms_by_sw, icols, scols = _schedule(prm, n_sl_u)

    # slot offset (in slices) of each cell in the uniform stream
    cell_sl = n_sl_u.reshape(ncell)
    cell_off = np.zeros(ncell, dtype=np.int64)
    np.cumsum(cell_sl[:-1], out=cell_off[1:])
    S_total = int(cell_sl.sum())

    # fill per-core gather-index / dst-local arrays
    gidx_all = np.zeros((C, P, icols), dtype=np.int16)
    dstl_all = np.full((C, P, scols), -1.0, dtype=np.float32)
    for c in range(C):
        pc = percore[c]
        ne = pc["cell"].shape[0]
        # rank within cell
        cc = counts[c]
        starts = np.zeros(ncell, dtype=np.int64)
        np.cumsum(cc[:-1], out=starts[1:])
        rank = np.arange(ne, dtype=np.int64) - starts[pc["cell"]]
        pos = cell_off[pc["cell"]] * P + rank  # global slot position
        vals = np.zeros(S_total * P, dtype=np.int16)
        dvals = np.full(S_total * P, -1.0, dtype=np.float32)
        vals[pos] = pc["rel"]
        dvals[pos] = pc["q"]
        # per-call packing
        for cm in calls:
            # slices of this call are contiguous in the stream
            sl0 = cm.scol
            seg = vals[(sl0) * P : (sl0 + cm.S) * P]
            gidx_all[c, :, cm.icol : cm.icol + 8 * cm.S] = _wrap_idx(seg)
            dstl_all[c, :, cm.scol : cm.scol + cm.S] = (
                dvals[(sl0) * P : (sl0 + cm.S) * P].reshape(cm.S, P).T
            )

    # phase-A inputs
    xT = np.zeros((FIN, prm.N2), dtype=np.float32)
    xT[:, :N] = x.T
    WT = np.ascontiguousarray(W.T)  # [FIN, FOUT]
    dinvA = np.zeros((P, prm.NG * prm.J), dtype=np.float32)
    n_idx = np.arange(prm.N2)
    g_i, j_i, p_i = n_idx // prm.WG, (n_idx % prm.WG) // P, n_idx % P
    dpad = np.zeros(prm.N2, dtype=np.float32)
    dpad[:N] = dinv
    dinvA[p_i, g_i * prm.J + j_i] = dpad
    iota = np.broadcast_to(
        np.arange(P, dtype=np.float32)[None, :], (P, P)
    ).copy()
    dinvD = np.zeros((C, P, prm.NSW * prm.TPSW), dtype=np.float32)
    w_idx = np.arange(prm.NSW * prm.TPSW)
    for c in range(C):
        node = c * NS + w_idx[:, None] * P + np.arange(P)[None, :]
        ok = node < (c + 1) * NS
        dv = np.where(ok, dinv[np.minimum(node, N - 1)], 0.0)
        dinvD[c][np.arange(P)[None, :], w_idx[:, None]] = dv

    inputs = []
    gpc = prm.NGpc  # phase-A shard: core c computes write groups [c*gpc, (c+1)*gpc)
    for c in range(C):
        inputs.append(
            {
                "xT": np.ascontiguousarray(
                    xT[:, c * gpc * prm.WG : (c + 1) * gpc * prm.WG]
                ),
                "WT": WT,
                "dinvA": np.ascontiguousarray(
                    dinvA[:, c * gpc * prm.J : (c + 1) * gpc * prm.J]
                ),
                "iota": iota,
                "dinvD": dinvD[c],
                "gidx": gidx_all[c],
                "dstl": dstl_all[c],
            }
        )
    return inputs, calls, mms_by_sw


def _split_sync_waits(nc):
    """This env's walrus rejects >1 sync wait on some opcodes; keep 1 wait
    per instruction, moving extras onto preceding same-engine NOPs."""
    for bb in nc.main_func.blocks:
        insts = bb.instructions
        i = 0
        while i < len(insts):
            ins = insts[i]
            si = ins.sync_info
            if si is not None and si.on_wait is not None and len(si.on_wait) > 1:
                waits = list(si.on_wait)
                keep, extra = waits[-1:], waits[:-1]
                k = 0
                while extra:
                    chunk, extra = extra[:1], extra[1:]
                    nop = mybir.InstNoOp(name=f"{ins.name}-ws{k}", ins=[], outs=[])
                    nop.engine = ins.engine
                    nop.sync_info = mybir.SyncInfo(on_wait=chunk, on_update=[])
                    nc.register_instruction(nop)
                    insts.insert(i, nop)
                    i += 1
                    k += 1
                ins.sync_info = mybir.SyncInfo(
                    on_wait=keep, on_update=list(si.on_update or [])
                )
            i += 1


def _build_program(prm, calls, mms_by_sw, icols, scols, mode="full"):
    f32 = mybir.dt.float32
    nc = bacc.Bacc("TRN2", num_swdge_queues=4)

    NGpc = prm.NGpc
    xT = nc.declare_dram_parameter(
        "xT", [FIN, NGpc * prm.WG], f32, isOutput=False
    )
    WT = nc.declare_dram_parameter("WT", [FIN, FOUT], f32, isOutput=False)
    dinvA = nc.declare_dram_parameter(
        "dinvA", [P, NGpc * prm.J], f32, isOutput=False
    )
    iota = nc.declare_dram_parameter("iota", [P, P], f32, isOutput=False)
    dinvD = nc.declare_dram_parameter(
        "dinvD", [P, prm.NSW * prm.TPSW], f32, isOutput=False
    )
    gidx = nc.declare_dram_parameter("gidx", [P, icols], mybir.dt.int16, isOutput=False)
    dstl = nc.declare_dram_parameter("dstl", [P, scols], f32, isOutput=False)
    y = nc.declare_dram_parameter("y", [prm.NS, FOUT], f32, isOutput=True)
    # phase A is sharded: each core computes its table shard, then an
    # AllGather assembles the full table every core gathers from
    TBSH = nc.dram_tensor("tbsh", [NGpc * prm.WG, FOUT], f32)
    TBL = nc.dram_tensor("tbl", [prm.TBLR, FOUT], f32, addr_space="Shared")

    with tile.TileContext(nc) as tc:
        with tc.tile_pool(name="const", bufs=1) as cpool:
            wt_sb = cpool.tile([FIN, FOUT], f32, tag="wt")
            nc.sync.dma_start(out=wt_sb[:], in_=WT[:])
            dinvA_sb = cpool.tile([P, NGpc * prm.J], f32, tag="da")
            nc.sync.dma_start(out=dinvA_sb[:], in_=dinvA[:])
            iota_sb = cpool.tile([P, P], f32, tag="io")
            nc.sync.dma_start(out=iota_sb[:], in_=iota[:])
            dinvD_sb = cpool.tile([P, prm.NSW * prm.TPSW], f32, tag="dd")
            nc.sync.dma_start(out=dinvD_sb[:], in_=dinvD[:])

            # ---------------- Phase A: build the h~ table shard -----------
            with (
                tc.tile_pool(name="pa", bufs=3) as pa,
                tc.tile_pool(name="psa", bufs=2, space="PSUM") as psa,
            ):
                for g in range(NGpc):
                    xt = pa.tile([P, prm.WG], f32, tag="xt")
                    nc.sync.dma_start(out=xt[:], in_=xT[:, g * prm.WG : (g + 1) * prm.WG])
                    hps = psa.tile([P, prm.J * FOUT], f32, tag="hps")
                    for j in range(prm.J):
                        nc.tensor.matmul(
                            out=hps[:, j * FOUT : (j + 1) * FOUT],
                            lhsT=xt[:, j * P : (j + 1) * P],
                            rhs=wt_sb[:],
                            start=True,
                            stop=True,
                        )
                    tsb = pa.tile([P, prm.J, FOUT], f32, tag="tsb")
                    nc.vector.tensor_tensor(
                        out=tsb[:],
                        in0=hps[:].rearrange("p (j f) -> p j f", f=FOUT),
                        in1=dinvA_sb[:, g * prm.J : (g + 1) * prm.J][
                            :, :, None
                        ].to_broadcast([P, prm.J, FOUT]),
                        op=mybir.AluOpType.mult,
                    )
                    base = prm.WG * g
                    nc.sync.dma_start(
                        out=TBSH[base : base + prm.WG, :].rearrange(
                            "(p j) f -> p j f", j=prm.J
                        ),
                        in_=tsb[:],
                    )

            # assemble the full table from all cores' shards
            nc.gpsimd.collective_compute(
                "AllGather",
                mybir.AluOpType.bypass,
                replica_groups=[list(range(prm.C))],
                ins=[TBSH[:]],
                outs=[TBL[:]],
            )

            # ---------------- Phase B: gather + segment-sum ----------------
            if mode == "phaseA":
                with tc.tile_pool(name="pz", bufs=1) as pz:
                    zy = pz.tile([P, FOUT], f32, tag="zy")
                    nc.vector.memset(zy[:], 0.0)
                    for r0 in range(0, prm.NS, P):
                        rt = min(P, prm.NS - r0)
                        nc.sync.dma_start(out=y[r0 : r0 + rt, :], in_=zy[:rt, :])
                calls = []
            S_MAX = max((cm.S for cm in calls), default=1)
            calls_by_sw = [[] for _ in range(prm.NSW)]
            for cm in calls:
                calls_by_sw[cm.sw].append(cm)
            nbufs = 2 * sum(1 for cm in calls_by_sw[0]) if calls_by_sw else 8
            nbufs = max(9, min(9, nbufs))
            qctr = [0]
            with (
                tc.tile_pool(name="pidx", bufs=nbufs) as pidx,
                tc.tile_pool(name="pg", bufs=nbufs) as pg,
                tc.tile_pool(name="pb", bufs=nbufs) as pb,
                tc.tile_pool(name="py", bufs=2) as py,
                tc.tile_pool(name="psb", bufs=2, space="PSUM") as psb,
            ):
                for sw in range(prm.NSW):
                    if not calls_by_sw[sw]:
                        continue
                    tiles = {}  # (bk, k) -> (g_t, b_t)
                    for cm in calls_by_sw[sw]:
                        S = cm.S
                        idx_t = pidx.tile([P, 8 * S_MAX], mybir.dt.int16, tag="idx")
                        nc.sync.dma_start(
                            out=idx_t[:, : 8 * S],
                            in_=gidx[:, cm.icol : cm.icol + 8 * S],
                        )
                        dst_t = pidx.tile([P, S_MAX], f32, tag="dst")
                        nc.sync.dma_start(
                            out=dst_t[:, :S], in_=dstl[:, cm.scol : cm.scol + S]
                        )
                        g_t = pg.tile([P, S_MAX, FOUT], f32, tag="g")
                        if mode == "nogather":
                            nc.vector.memset(g_t[:, :S, :], 0.0)
                        else:
                            nc.gpsimd.dma_gather(
                                out_ap=g_t[:, :S, :],
                                in_ap=TBL[
                                    cm.bk * prm.BKCAP : min(
                                        (cm.bk + 1) * prm.BKCAP, prm.TBLR
                                    ),
                                    :,
                                ],
                                idxs_ap=idx_t[:, : 8 * S],
                                num_idxs=S * P,
                                num_idxs_reg=S * P,
                                elem_size=FOUT,
                                single_packet=False,
                                queue_num=qctr[0] % 4,
                            )
                            qctr[0] += 1
                        b_t = pb.tile([P, S_MAX, P], f32, tag="b")
                        nc.vector.tensor_tensor(
                            out=b_t[:, :S, :],
                            in0=dst_t[:, :S][:, :, None].to_broadcast([P, S, P]),
                            in1=iota_sb[:, None, :].to_broadcast([P, S, P]),
                            op=mybir.AluOpType.is_equal,
                        )
                        tiles[(cm.bk, cm.k)] = (g_t, b_t)
                    psum_t = [
                        psb.tile([P, FOUT], f32, tag=f"acc{t}", name=f"acc{t}")
                        for t in range(prm.TPSW)
                    ]
                    for bk, s, t, st, sp in mms_by_sw[sw]:
                        g_t, b_t = tiles[(bk, s // prm.S_CAP)]
                        sl = s % prm.S_CAP
                        nc.tensor.matmul(
                            out=psum_t[t][:],
                            lhsT=b_t[:, sl, :],
                            rhs=g_t[:, sl, :],
                            start=st,
                            stop=sp,
                        )
                    # scale by dinv[dst] (on the otherwise-idle Scalar engine,
                    # keeping DVE free for the next superwindow's B-builds)
                    rows_sw = min(prm.SWD, prm.NS - sw * prm.SWD)
                    nt = (rows_sw + P - 1) // P  # valid dst tiles this sw
                    ysb = py.tile([P, prm.TPSW, FOUT], f32, tag="ysb")
                    for t in range(nt):
                        w = sw * prm.TPSW + t
                        nc.scalar.activation(
                            out=ysb[:, t, :],
                            in_=psum_t[t][:],
                            func=mybir.ActivationFunctionType.Copy,
                            scale=dinvD_sb[:, w : w + 1],
                        )
                    for t in range(nt):
                        rt = min(P, rows_sw - t * P)
                        r0 = sw * prm.SWD + t * P
                        nc.sync.dma_start(out=y[r0 : r0 + rt, :], in_=ysb[:rt, t, :])

    nc.compile()
    _split_sync_waits(nc)
    return nc


_CACHE = {}


def _get_program_and_prep(x, edge_index, W, prm):
    inputs, calls, mms_by_sw = _host_prep(x, edge_index, W, prm)
    icols = sum(8 * cm.S for cm in calls)
    scols = sum(cm.S for cm in calls)
    nc = _build_program(prm, calls, mms_by_sw, icols, scols)
    return nc, inputs


def kernel(x, edge_index, W):
    prm = Prm(N=int(x.shape[0]))
    nc, inputs = _get_program_and_prep(x, edge_index, W, prm)
    res = run_bass_kernel_spmd(nc, inputs, list(range(prm.C)))
    y = np.concatenate([res.results[c]["y"] for c in range(prm.C)], axis=0)
    return y.astype(np.float32)


def run_with_trace(x, edge_index, W, trace_cores=None):
    """test.py helper: returns (y, BassKernelResults) with profiling."""
    prm = Prm(N=int(x.shape[0]))
    nc, inputs = _get_program_and_prep(x, edge_index, W, prm)
    res = run_bass_kernel_spmd(
        nc, inputs, list(range(prm.C)), trace=True, trace_cores=trace_cores
    )
    y = np.concatenate([res.results[c]["y"] for c in range(prm.C)], axis=0)
    return y.astype(np.float32), res
